# revision 7
# baseline (speedup 1.0000x reference)
"""Bahdanau-style attention kernel for Trainium2, SPMD over 8 NeuronCores.

Reference computation (per batch b):
    att1 = encoder_out @ W_enc + b_enc            # [L, A]
    att2 = decoder_hidden @ W_dec + b_dec         # [A]
    att  = relu(att1 + att2) @ W_full + b_full    # [L]
    alpha = softmax(att)                          # [L]
    weighted = alpha @ encoder_out                # [E]
returns (weighted [B, E], alpha [B, L])

Sharding: data-parallel over batch B=128 across 8 cores (16 batches/core);
weights replicated.  The host feeds each core encoder_out already
transposed to [16, E, L] ("encT") so that the contraction dim E lands on
SBUF partitions (TensorE contracts the partition dim; fp32 DMA-transpose
does not exist on-device).  b_full is dropped: softmax is shift-invariant.

Matmul dtype: float32r (single-pass fp32 matmul, 1 cycle/row) vs float32
(2-pass, 4 cycles/row).  For float32r, walrus requires every matmul input
to be produced in float32r; we declare the relevant DRAM inputs as f32r
(bit-identical to f32 host-side) and have ACT write f32r tiles on-chip.

Device pipeline per batch:
  PE   : att1T[A,L] accumulated over E chunks (lhsT=W_enc chunk, rhs=encT chunk)
  ACT  : relu_t = Relu(att1T + (att2+b_enc+b_dec)[a]) -- bias is per-partition
  PE   : att[1,L] = W_full.T @ relu_t
  ACT  : exp = Exp(att), with fused free-dim sum (accum_out)
  DVE  : inv = 1/sum ; ACT: alpha = exp * inv (f32 for output, f32r for bcast)
  PE   : alpha_rep[128,L] = ones[1,128].T @ alpha   (partition broadcast)
  DVE  : weighted chunks via tensor_tensor_reduce(encT * alpha_rep) -> [128,1]x4
  PE   : transpose [128,4] -> [4,128] so the DRAM write is contiguous
"""
import os
import numpy as np
from contextlib import ExitStack

import concourse.bass as bass
import concourse.bacc as bacc
import concourse.tile as tile
from concourse import mybir, bass_utils

B, L, E, D, A = 128, 1024, 512, 512, 512
NCORES = 8
BL = B // NCORES  # batches per core

F32 = mybir.dt.float32
_MM_DT_NAME = os.environ.get("KERNEL_MM_DTYPE", "float32r")
MM_DT = getattr(mybir.dt, _MM_DT_NAME)
AF = mybir.ActivationFunctionType
ALU = mybir.AluOpType

# consts (f32) layout [128, 132]: [:,0:128] identity ; [:,128:132] bias chunks
CN_EYE, CN_BIAS, CN_W = 0, 128, 132
# wfo (matmul dtype) layout [128, 132]:
#   [:,0:4] W_full column-chunks (wfo[p, c] = W_full[128c+p])
#   [0,4:132] row of 128 ones (lhsT of the partition-broadcast outer product)
WF_WF, WF_ONES, WF_W = 0, 4, 132


def ts(i, n):
    return slice(i * n, (i + 1) * n)


def build_nc(compile=True):
    nc = bacc.Bacc("TRN2", target_bir_lowering=False, debug=False,
                   num_devices=NCORES)
    encT = nc.dram_tensor("enct", [BL, E, L], MM_DT, kind="ExternalInput").ap()
    decT = nc.dram_tensor("dect", [D, BL], F32, kind="ExternalInput").ap()
    wenc = nc.dram_tensor("wenc", [E, A], MM_DT, kind="ExternalInput").ap()
    wdec = nc.dram_tensor("wdec", [D, A], F32, kind="ExternalInput").ap()
    consts = nc.dram_tensor("consts", [128, CN_W], F32, kind="ExternalInput").ap()
    wfo = nc.dram_tensor("wfo", [128, WF_W], MM_DT, kind="ExternalInput").ap()
    w_out = nc.dram_tensor("weighted", [BL, E], F32, kind="ExternalOutput").ap()
    a_out = nc.dram_tensor("alpha", [BL, L], F32, kind="ExternalOutput").ap()

    nE, nA, nD = E // 128, A // 128, D // 128  # 4 chunks each
    nLh = 2  # L in two 512-wide halves (fp32 moving-operand max / PSUM bank)

    with ExitStack() as ctx:
        tc = ctx.enter_context(tile.TileContext(nc))
        cpool = ctx.enter_context(tc.tile_pool(name="cpool", bufs=1))

        consts_t = cpool.tile([128, CN_W], F32, name="consts_t", tag="consts_t")
        nc.sync.dma_start(consts_t[:], consts[:])
        wfo_t = cpool.tile([128, WF_W], MM_DT, name="wfo_t", tag="wfo_t")
        nc.sync.dma_start(wfo_t[:], wfo[:])

        we = []
        for ec in range(nE):
            t = cpool.tile([128, A], MM_DT, name=f"we{ec}", tag=f"we{ec}")
            nc.sync.dma_start(t[:], wenc[ts(ec, 128), :])
            we.append(t)
        wd = []
        for dc in range(nD):
            t = cpool.tile([128, A], F32, name=f"wd{dc}", tag=f"wd{dc}")
            nc.sync.dma_start(t[:], wdec[ts(dc, 128), :])
            wd.append(t)
        dT = []
        for dc in range(nD):
            t = cpool.tile([128, BL], F32, name=f"dT{dc}", tag=f"dT{dc}")
            nc.sync.dma_start(t[:], decT[ts(dc, 128), :])
            dT.append(t)

        # att2T[a, b] = sum_d W_dec[d, a] * decT[d, b], plus combined bias.
        # Tiny one-time cost; stays plain fp32 matmul.
        att2 = []
        with tc.tile_pool(name="a2ps", bufs=1, space="PSUM") as a2pool:
            for ac in range(nA):
                ps = a2pool.tile([128, BL], F32, name=f"a2ps{ac}", tag="a2ps")
                for dc in range(nD):
                    nc.tensor.matmul(ps[:], wd[dc][:, ts(ac, 128)], dT[dc][:],
                                     start=(dc == 0), stop=(dc == nD - 1))
                sb = cpool.tile([128, BL], F32, name=f"att2_{ac}",
                                tag=f"att2_{ac}")
                nc.scalar.activation(sb[:], ps[:], AF.Identity,
                                     bias=consts_t[:, CN_BIAS + ac:CN_BIAS + ac + 1],
                                     scale=1.0)
                att2.append(sb)

        etp = ctx.enter_context(tc.tile_pool(name="etp", bufs=3))
        rlp = ctx.enter_context(tc.tile_pool(name="rlp", bufs=2))
        scp = ctx.enter_context(tc.tile_pool(name="scp", bufs=2))
        smp = ctx.enter_context(tc.tile_pool(name="smp", bufs=3))
        psmm = ctx.enter_context(tc.tile_pool(name="psmm", bufs=3, space="PSUM"))
        psatt = ctx.enter_context(tc.tile_pool(name="psatt", bufs=1, space="PSUM"))
        psrep = ctx.enter_context(tc.tile_pool(name="psrep", bufs=1, space="PSUM"))
        pswt = ctx.enter_context(tc.tile_pool(name="pswt", bufs=1, space="PSUM"))

        for b in range(BL):
            et = []
            for ec in range(nE):
                t = etp.tile([128, L], MM_DT, name=f"et{ec}", tag=f"et{ec}")
                nc.sync.dma_start(t[:], encT[b, ts(ec, 128), :])
                et.append(t)

            # att1T chunks + fused bias/relu
            rl = []
            for ac in range(nA):
                r = rlp.tile([128, L], MM_DT, name=f"rl{ac}", tag=f"rl{ac}")
                rl.append(r)
                for lh in range(nLh):
                    ps = psmm.tile([128, 512], F32, name="ps1", tag="ps1")
                    for ec in range(nE):
                        nc.tensor.matmul(ps[:], we[ec][:, ts(ac, 128)],
                                         et[ec][:, ts(lh, 512)],
                                         start=(ec == 0), stop=(ec == nE - 1))
                    nc.scalar.activation(r[:, ts(lh, 512)], ps[:], AF.Relu,
                                         bias=att2[ac][:, b:b + 1], scale=1.0)

            # attention scores att[1, L]
            att_ps = psatt.tile([1, L], F32, name="attps", tag="attps")
            for lh in range(nLh):
                for ac in range(nA):
                    nc.tensor.matmul(att_ps[0:1, ts(lh, 512)],
                                     wfo_t[:, WF_WF + ac:WF_WF + ac + 1],
                                     rl[ac][:, ts(lh, 512)],
                                     start=(ac == 0), stop=(ac == nA - 1))

            # softmax on partition 0 (no max-subtraction: logits are O(1))
            exp_sb = smp.tile([1, L], F32, name="exp_sb", tag="exp_sb")
            ssum = smp.tile([1, 1], F32, name="ssum", tag="ssum")
            nc.scalar.activation(exp_sb[:], att_ps[:], AF.Exp,
                                 accum_out=ssum[:])
            inv = smp.tile([1, 1], F32, name="inv", tag="inv")
            nc.vector.reciprocal(inv[:], ssum[:])
            alpha_sb = smp.tile([1, L], F32, name="alpha_sb", tag="alpha_sb")
            nc.scalar.mul(alpha_sb[:], exp_sb[:], inv[0:1, 0:1])
            nc.sync.dma_start(a_out[b, :], alpha_sb[0:1, :])
            if MM_DT != F32:
                alpha_r = smp.tile([1, L], MM_DT, name="alpha_r", tag="alpha_r")
                nc.scalar.mul(alpha_r[:], exp_sb[:], inv[0:1, 0:1])
            else:
                alpha_r = alpha_sb

            # broadcast alpha to all partitions: ones[1,128].T @ alpha[1,512]
            arep = psrep.tile([128, L], F32, name="arep", tag="arep")
            for lh in range(nLh):
                nc.tensor.matmul(arep[:, ts(lh, 512)],
                                 wfo_t[0:1, WF_ONES:WF_ONES + 128],
                                 alpha_r[0:1, ts(lh, 512)],
                                 start=True, stop=True)

            # weighted[e] = sum_l encT[e, l] * alpha[l]
            # (tensor_tensor_reduce would fuse these, but custom DVE ops
            #  hard-fault in this environment; use standard TT + reduce.
            #  DVE TT reading PSUM directly also faults -> SBUF copy first.)
            arep_sb = scp.tile([128, L], F32, name="arep_sb", tag="arep_sb")
            nc.vector.tensor_copy(arep_sb[:], arep[:])
            wsum = smp.tile([128, nE], F32, name="wsum", tag="wsum")
            for ec in range(nE):
                sc = scp.tile([128, L], F32, name="ttr_out", tag="ttr_out")
                nc.vector.tensor_tensor(sc[:], et[ec][:].bitcast(F32),
                                        arep_sb[:], ALU.mult)
                nc.vector.reduce_sum(wsum[:, ec:ec + 1], sc[:],
                                     axis=mybir.AxisListType.X)

            # [128, 4] -> [4, 128] so DRAM rows are contiguous
            wt = pswt.tile([nE, 128], F32, name="wt", tag="wt")
            nc.tensor.transpose(wt[:], wsum[:], consts_t[:, CN_EYE:CN_EYE + 128])
            wrow = smp.tile([nE, 128], F32, name="wrow", tag="wrow")
            nc.vector.tensor_copy(wrow[:], wt[:])
            nc.sync.dma_start(w_out[b].rearrange("(c p) -> c p", c=nE),
                              wrow[:])
    if compile:
        nc.compile()
    return nc


_NC_CACHE = None


def _get_nc():
    global _NC_CACHE
    if _NC_CACHE is None:
        _NC_CACHE = build_nc()
    return _NC_CACHE


def make_in_maps(encoder_out, decoder_hidden, W_enc, b_enc, W_dec, b_dec,
                 W_full, b_full):
    enc = np.ascontiguousarray(np.asarray(encoder_out, dtype=np.float32))
    dec = np.asarray(decoder_hidden, dtype=np.float32)
    W_enc = np.ascontiguousarray(np.asarray(W_enc, dtype=np.float32))
    W_dec = np.ascontiguousarray(np.asarray(W_dec, dtype=np.float32))
    bias = (np.asarray(b_enc, dtype=np.float32)
            + np.asarray(b_dec, dtype=np.float32))
    W_full = np.asarray(W_full, dtype=np.float32)

    consts = np.zeros((128, CN_W), dtype=np.float32)
    consts[:, CN_EYE:CN_EYE + 128] = np.eye(128, dtype=np.float32)
    consts[:, CN_BIAS:CN_BIAS + 4] = bias.reshape(4, 128).T
    wfo = np.zeros((128, WF_W), dtype=np.float32)
    wfo[:, WF_WF:WF_WF + 4] = W_full.reshape(4, 128).T
    wfo[0, WF_ONES:WF_ONES + 128] = 1.0

    in_maps = []
    for c in range(NCORES):
        sl = slice(c * BL, (c + 1) * BL)
        in_maps.append({
            "enct": np.ascontiguousarray(enc[sl].transpose(0, 2, 1)),
            "dect": np.ascontiguousarray(dec[sl].T),
            "wenc": W_enc,
            "wdec": W_dec,
            "consts": consts,
            "wfo": wfo,
        })
    return in_maps


def kernel(**inputs):
    in_maps = make_in_maps(**inputs)
    nc = _get_nc()
    res = bass_utils.run_bass_kernel_spmd(nc, in_maps, list(range(NCORES)))
    weighted = np.concatenate([res.results[c]["weighted"]
                               for c in range(NCORES)], axis=0)
    alpha = np.concatenate([res.results[c]["alpha"]
                            for c in range(NCORES)], axis=0)
    return weighted, alpha


# revision 18
# speedup vs baseline: 2.7030x; 2.7030x over previous
"""Bahdanau-style attention kernel for Trainium2, SPMD over 8 NeuronCores.

Reference computation (per batch b):
    att1 = encoder_out @ W_enc + b_enc            # [L, A]
    att2 = decoder_hidden @ W_dec + b_dec         # [A]
    att  = relu(att1 + att2) @ W_full + b_full    # [L]
    alpha = softmax(att)                          # [L]
    weighted = alpha @ encoder_out                # [E]
returns (weighted [B, E], alpha [B, L])

Sharding: data-parallel over batch B=128 across 8 cores (16 batches/core);
weights replicated.  The host feeds each core encoder_out already
transposed to [16, E, L] ("encT") so that the contraction dim E lands on
SBUF partitions (TensorE contracts the partition dim; fp32 DMA-transpose
does not exist on-device).  b_full is dropped: softmax is shift-invariant.

Matmul dtype: float32r (single-pass fp32 matmul, 1 cycle/row) vs float32
(2-pass, 4 cycles/row).  For float32r, walrus requires every matmul input
to be produced in float32r; we declare the relevant DRAM inputs as f32r
(bit-identical to f32 host-side) and have ACT write f32r tiles on-chip.

Device pipeline per batch:
  PE   : att1T[A,L] accumulated over E chunks (lhsT=W_enc chunk, rhs=encT chunk)
  ACT  : relu_t = Relu(att1T + (att2+b_enc+b_dec)[a]) -- bias is per-partition
  PE   : att[1,L] = W_full.T @ relu_t
  ACT  : exp = Exp(att), with fused free-dim sum (accum_out)
  DVE  : inv = 1/sum ; ACT: alpha = exp * inv (written once in f32r; the
         same bits serve the f32 alpha output via bitcast)
  PE   : alpha_rep[128,L] = ones[1,128].T @ alpha   (partition broadcast)
  DVE  : copy alpha_rep PSUM->SBUF (DVE reading PSUM in tensor_tensor and
         all custom DVE ops, e.g. tensor_tensor_reduce, hard-fault here)
  DVE/GPSIMD : products encT_chunk * alpha_rep (split across both engines)
  DVE  : reduce_sum -> weighted chunk columns of a [128, 64] accumulator
  PE   : one transpose [128,64] -> [64,128] at the end; single contiguous
         DMA writes all 16 batches' weighted rows
"""
import os
import numpy as np
from contextlib import ExitStack

import concourse.bass as bass
import concourse.bacc as bacc
import concourse.tile as tile
from concourse import mybir, bass_utils

B, L, E, D, A = 128, 1024, 512, 512, 512
NCORES = 8
BL = B // NCORES  # batches per core

F32 = mybir.dt.float32
_MM_DT_NAME = os.environ.get("KERNEL_MM_DTYPE", "float32r")
MM_DT = getattr(mybir.dt, _MM_DT_NAME)
# how many of the 4 weighted-sum products run on DVE (rest on GPSIMD)
N_TT_DVE = int(os.environ.get("KERNEL_N_TT_DVE", "2"))
# repeat the whole batch loop K times (for amortized HW timing only)
REPS = int(os.environ.get("KERNEL_REPS", "1"))
AF = mybir.ActivationFunctionType
ALU = mybir.AluOpType

# consts (f32) layout [128, 132]: [:,0:128] identity ; [:,128:132] bias chunks
CN_EYE, CN_BIAS, CN_W = 0, 128, 132
# wfo (matmul dtype) layout [128, 132]:
#   [:,0:4] W_full column-chunks (wfo[p, c] = W_full[128c+p])
#   [0,4:132] row of 128 ones (lhsT of the partition-broadcast outer product)
WF_WF, WF_ONES, WF_W = 0, 4, 132


def ts(i, n):
    return slice(i * n, (i + 1) * n)


def build_nc(compile=True):
    nc = bacc.Bacc("TRN2", target_bir_lowering=False, debug=False,
                   num_devices=NCORES)
    encT = nc.dram_tensor("enct", [BL, E, L], MM_DT, kind="ExternalInput").ap()
    decT = nc.dram_tensor("dect", [D, BL], F32, kind="ExternalInput").ap()
    wenc = nc.dram_tensor("wenc", [E, A], MM_DT, kind="ExternalInput").ap()
    wdec = nc.dram_tensor("wdec", [D, A], F32, kind="ExternalInput").ap()
    consts = nc.dram_tensor("consts", [128, CN_W], F32, kind="ExternalInput").ap()
    wfo = nc.dram_tensor("wfo", [128, WF_W], MM_DT, kind="ExternalInput").ap()
    w_out = nc.dram_tensor("weighted", [BL, E], F32, kind="ExternalOutput").ap()
    a_out = nc.dram_tensor("alpha", [BL, L], F32, kind="ExternalOutput").ap()

    nE, nA, nD = E // 128, A // 128, D // 128  # 4 chunks each
    nLh = 2  # L in two 512-wide halves (fp32 moving-operand max / PSUM bank)

    with ExitStack() as ctx:
        tc = ctx.enter_context(tile.TileContext(nc))
        cpool = ctx.enter_context(tc.tile_pool(name="cpool", bufs=1))

        consts_t = cpool.tile([128, CN_W], F32, name="consts_t", tag="consts_t")
        nc.sync.dma_start(consts_t[:], consts[:])
        wfo_t = cpool.tile([128, WF_W], MM_DT, name="wfo_t", tag="wfo_t")
        nc.sync.dma_start(wfo_t[:], wfo[:])

        we = []
        for ec in range(nE):
            t = cpool.tile([128, A], MM_DT, name=f"we{ec}", tag=f"we{ec}")
            nc.sync.dma_start(t[:], wenc[ts(ec, 128), :])
            we.append(t)

        etp = ctx.enter_context(tc.tile_pool(name="etp", bufs=4))
        rlp = ctx.enter_context(tc.tile_pool(name="rlp", bufs=3))
        scp = ctx.enter_context(tc.tile_pool(name="scp", bufs=2))
        smp = ctx.enter_context(tc.tile_pool(name="smp", bufs=3))
        psmm = ctx.enter_context(tc.tile_pool(name="psmm", bufs=4, space="PSUM"))
        psatt = ctx.enter_context(tc.tile_pool(name="psatt", bufs=1, space="PSUM"))
        psrep = ctx.enter_context(tc.tile_pool(name="psrep", bufs=1, space="PSUM"))

        def load_et(b):
            out = []
            for ec in range(nE):
                t = etp.tile([128, L], MM_DT, name=f"et{ec}", tag=f"et{ec}")
                nc.sync.dma_start(t[:], encT[b, ts(ec, 128), :])
                out.append(t)
            return out

        # prefetch batch 0 before the (serial) att2 setup below
        et0 = load_et(0)

        wd = []
        for dc in range(nD):
            t = cpool.tile([128, A], F32, name=f"wd{dc}", tag=f"wd{dc}")
            nc.sync.dma_start(t[:], wdec[ts(dc, 128), :])
            wd.append(t)
        dT = []
        for dc in range(nD):
            t = cpool.tile([128, BL], F32, name=f"dT{dc}", tag=f"dT{dc}")
            nc.sync.dma_start(t[:], decT[ts(dc, 128), :])
            dT.append(t)

        # att2T[a, b] = sum_d W_dec[d, a] * decT[d, b], plus combined bias.
        # Tiny one-time cost; stays plain fp32 matmul.
        att2 = []
        if True:
            for ac in range(nA):
                ps = psmm.tile([128, BL], F32, name=f"a2ps{ac}", tag="ps1")
                for dc in range(nD):
                    nc.tensor.matmul(ps[:], wd[dc][:, ts(ac, 128)], dT[dc][:],
                                     start=(dc == 0), stop=(dc == nD - 1))
                sb = cpool.tile([128, BL], F32, name=f"att2_{ac}",
                                tag=f"att2_{ac}")
                nc.scalar.activation(sb[:], ps[:], AF.Identity,
                                     bias=consts_t[:, CN_BIAS + ac:CN_BIAS + ac + 1],
                                     scale=1.0)
                att2.append(sb)

        # weighted-sum accumulator for all batches: column 4b+c holds
        # chunk c of batch b; one transpose + one DMA at the end.
        wsum_all = cpool.tile([128, nE * BL], F32, name="wsum_all",
                              tag="wsum_all")

        for rep, b in [(r, bb) for r in range(REPS) for bb in range(BL)]:
            et = et0 if (rep == 0 and b == 0) else load_et(b)

            # att1T chunks + fused bias/relu
            rl = []
            for ac in range(nA):
                r = rlp.tile([128, L], MM_DT, name=f"rl{ac}", tag=f"rl{ac}")
                rl.append(r)
                for lh in range(nLh):
                    ps = psmm.tile([128, 512], F32, name="ps1", tag="ps1")
                    for ec in range(nE):
                        nc.tensor.matmul(ps[:], we[ec][:, ts(ac, 128)],
                                         et[ec][:, ts(lh, 512)],
                                         start=(ec == 0), stop=(ec == nE - 1))
                    nc.scalar.activation(r[:, ts(lh, 512)], ps[:], AF.Relu,
                                         bias=att2[ac][:, b:b + 1], scale=1.0)

            # attention scores att[1, L]
            att_ps = psatt.tile([1, L], F32, name="attps", tag="attps")
            for lh in range(nLh):
                for ac in range(nA):
                    nc.tensor.matmul(att_ps[0:1, ts(lh, 512)],
                                     wfo_t[:, WF_WF + ac:WF_WF + ac + 1],
                                     rl[ac][:, ts(lh, 512)],
                                     start=(ac == 0), stop=(ac == nA - 1))

            # softmax on partition 0 (no max-subtraction: logits are O(1))
            exp_sb = smp.tile([1, L], F32, name="exp_sb", tag="exp_sb")
            ssum = smp.tile([1, 1], F32, name="ssum", tag="ssum")
            nc.scalar.activation(exp_sb[:], att_ps[:], AF.Exp,
                                 accum_out=ssum[:])
            inv = smp.tile([1, 1], F32, name="inv", tag="inv")
            nc.vector.reciprocal(inv[:], ssum[:])
            alpha_r = smp.tile([1, L], MM_DT, name="alpha_r", tag="alpha_r")
            nc.scalar.mul(alpha_r[:], exp_sb[:], inv[0:1, 0:1])
            nc.sync.dma_start(a_out[b, :], alpha_r[0:1, :].bitcast(F32))

            # broadcast alpha to all partitions: ones[1,128].T @ alpha[1,512]
            arep = psrep.tile([128, L], F32, name="arep", tag="arep")
            for lh in range(nLh):
                nc.tensor.matmul(arep[:, ts(lh, 512)],
                                 wfo_t[0:1, WF_ONES:WF_ONES + 128],
                                 alpha_r[0:1, ts(lh, 512)],
                                 start=True, stop=True)

            # weighted[e] = sum_l encT[e, l] * alpha[l]
            # (tensor_tensor_reduce would fuse these, but custom DVE ops
            #  hard-fault in this environment; use standard TT + reduce.
            #  DVE TT reading PSUM directly also faults -> SBUF copy first.)
            # Work is split across DVE / GPSIMD (products) and DVE / ACT
            # (reductions) to balance engine load; PE is the bottleneck.
            arep_sb = scp.tile([128, L], F32, name="arep_sb", tag="arep_sb")
            nc.vector.tensor_copy(arep_sb[:], arep[:])
            for ec in range(nE):
                sc = scp.tile([128, L], F32, name="ttr_out",
                              tag=f"ttr_out{ec % 2}")
                eng = nc.vector if ec < N_TT_DVE else nc.gpsimd
                eng.tensor_tensor(sc[:], et[ec][:].bitcast(F32),
                                  arep_sb[:], ALU.mult)
                nc.vector.reduce_sum(wsum_all[:, nE * b + ec:nE * b + ec + 1],
                                     sc[:], axis=mybir.AxisListType.X)

        # one transpose of the accumulated [128, 64] -> [64, 128]; DRAM
        # layout is then contiguous: partition 4b+c -> weighted[b, 128c:]
        if True:
            wt = psmm.tile([nE * BL, 128], F32, name="wt", tag="ps1")
            nc.tensor.transpose(wt[:], wsum_all[:],
                                consts_t[:, CN_EYE:CN_EYE + 128])
            wrow = smp.tile([nE * BL, 128], F32, name="wrow", tag="wrow")
            nc.vector.tensor_copy(wrow[:], wt[:])
            nc.sync.dma_start(
                w_out.rearrange("b (c p) -> (b c) p", c=nE), wrow[:])
    if compile:
        nc.compile()
    return nc


_NC_CACHE = None


def _get_nc():
    global _NC_CACHE
    if _NC_CACHE is None:
        _NC_CACHE = build_nc()
    return _NC_CACHE


def make_in_maps(encoder_out, decoder_hidden, W_enc, b_enc, W_dec, b_dec,
                 W_full, b_full):
    enc = np.ascontiguousarray(np.asarray(encoder_out, dtype=np.float32))
    dec = np.asarray(decoder_hidden, dtype=np.float32)
    W_enc = np.ascontiguousarray(np.asarray(W_enc, dtype=np.float32))
    W_dec = np.ascontiguousarray(np.asarray(W_dec, dtype=np.float32))
    bias = (np.asarray(b_enc, dtype=np.float32)
            + np.asarray(b_dec, dtype=np.float32))
    W_full = np.asarray(W_full, dtype=np.float32)

    consts = np.zeros((128, CN_W), dtype=np.float32)
    consts[:, CN_EYE:CN_EYE + 128] = np.eye(128, dtype=np.float32)
    consts[:, CN_BIAS:CN_BIAS + 4] = bias.reshape(4, 128).T
    wfo = np.zeros((128, WF_W), dtype=np.float32)
    wfo[:, WF_WF:WF_WF + 4] = W_full.reshape(4, 128).T
    wfo[0, WF_ONES:WF_ONES + 128] = 1.0

    in_maps = []
    for c in range(NCORES):
        sl = slice(c * BL, (c + 1) * BL)
        in_maps.append({
            "enct": np.ascontiguousarray(enc[sl].transpose(0, 2, 1)),
            "dect": np.ascontiguousarray(dec[sl].T),
            "wenc": W_enc,
            "wdec": W_dec,
            "consts": consts,
            "wfo": wfo,
        })
    return in_maps


def kernel(**inputs):
    in_maps = make_in_maps(**inputs)
    nc = _get_nc()
    res = bass_utils.run_bass_kernel_spmd(nc, in_maps, list(range(NCORES)))
    weighted = np.concatenate([res.results[c]["weighted"]
                               for c in range(NCORES)], axis=0)
    alpha = np.concatenate([res.results[c]["alpha"]
                            for c in range(NCORES)], axis=0)
    return weighted, alpha
